# revision 1
# baseline (speedup 1.0000x reference)
"""Trainium2 Bass kernel for Restormer-style transposed (channel) attention.

Math (per batch b):
  qkv = qkv_w @ x                    (1x1 conv, channel GEMM)
  qkv = DWConv3x3(qkv)               (per-channel 3x3, SAME zero pad)
  q,k,v = split(qkv); per head: q,k l2-normalized over spatial
  attn  = softmax((q_n @ k_n^T) * temperature)
  y     = proj_w @ (blockdiag(attn) @ v)

Sharding: 8 cores <-> (batch b = core//2, image-row half = core%2).
Cross-half reductions (channel sumsq + Gram) are tiny: one AllReduce of a
[128,200] f32 stat tile per pair. The final projection is folded with the
attention: y = (proj_w @ blockdiag(attn)) @ v, one 192x192 GEMM applied to v.

Engine layout per core:
  PE : qkv GEMM, q/k transposes, Gram, post matmuls, final GEMM
  DVE: depthwise conv (tensor_scalar + 8x scalar_tensor_tensor per tile),
       softmax small ops, final-GEMM evac
  ACT: PSUM->SBUF evacuations, sumsq (Square+accum), exp
  DMA: x streaming, odd-alignment shifted copy, v scratch, y store
"""

import sys
sys.path.insert(0, '/opt/trn_rl_repo')

import numpy as np
import ml_dtypes
from contextlib import ExitStack

import concourse.bass as bass
import concourse.bacc as bacc
import concourse.tile as tile
import concourse.mybir as mybir
from concourse.bass_utils import run_bass_kernel_spmd

bf16 = mybir.dt.bfloat16
f32 = mybir.dt.float32
Alu = mybir.AluOpType
Act = mybir.ActivationFunctionType

B, C, HEADS, CPH = 4, 192, 4, 48
W = 256
WP = W + 4          # 2 zero cols left, 2 right (keeps 4B alignment)
N_CORES = 8

# o-tile partition map over the 576 qkv channels (q:0-191, k:192-383, v:384-575)
OT = [(0, 128), (128, 128), (256, 128), (384, 128), (512, 64)]
TAPS = [(dy, dx) for dy in (-1, 0, 1) for dx in (-1, 0, 1)]

_BUILT = {}


def build(H=256, CR=16):
    """H: image height (256 real). CR: valid rows per chunk."""
    HALF = H // 2                      # image rows per core
    NCH = HALF // CR                   # chunks
    assert NCH * CR == HALF
    SH_ROWS = HALF + 2                 # shard rows incl. conv halo
    NF = (CR + 2) * WP                 # GEMM window elems per chunk
    NV = CR * W                        # valid conv-out elems per chunk
    NPX = HALF * W                     # valid pixels per core
    PXB = NV // 128                    # 128-px blocks per chunk
    assert PXB % 2 == 0
    VC = 1024                          # final-GEMM v chunk cols
    assert NPX % VC == 0

    nc = bacc.Bacc("TRN2", target_bir_lowering=False, debug=False,
                   num_devices=N_CORES)
    dram = lambda n, s, d, kind: nc.dram_tensor(n, s, d, kind=kind).ap()
    x_d = dram("x", [C, SH_ROWS * WP], bf16, "ExternalInput")
    wt1_d = dram("wt1", [128, 576], bf16, "ExternalInput")
    wt2_d = dram("wt2", [64, 576], bf16, "ExternalInput")
    wdw_d = dram("wdw", [128, 45], f32, "ExternalInput")
    pjt1_d = dram("pjt1", [128, 192], bf16, "ExternalInput")
    pjt2_d = dram("pjt2", [64, 192], bf16, "ExternalInput")
    id_d = dram("ident", [128, 128], bf16, "ExternalInput")
    id64_d = dram("ident64", [128, 64], bf16, "ExternalInput")
    tmp_d = dram("tempb", [48, 4], f32, "ExternalInput")
    y_d = dram("y", [C, NPX], f32, "ExternalOutput")
    dbg_d = dram("dbg", [128, 200], f32, "ExternalOutput")

    with tile.TileContext(nc) as tc, ExitStack() as ctx:
        P = lambda name, bufs, space="SBUF": ctx.enter_context(
            tc.tile_pool(name=name, bufs=bufs, space=space))
        wp = P("wp", 1)
        xp = P("xp", 2)
        qkvp = P("qkvp", 2)
        qsp = P("qsp", 2)
        cop = P("cop", 2)      # per-o-tile tags co0..co4
        sqp = P("sqp", 1)
        qtp = P("qtp", 2)
        stp = P("stp", 1)
        postp = P("postp", 1)
        vcp = P("vcp", 2)
        ysp = P("ysp", 2)
        ps_g = P("ps_g", 2, "PSUM")
        ps_t = P("ps_t", 1, "PSUM")
        ps_gram = P("ps_gram", 1, "PSUM")
        ps_post = P("ps_post", 2, "PSUM")
        drp = P("drp", 1, "DRAM")

        # ---- weights / constants in SBUF
        wt1 = wp.tile([128, 576], bf16, tag="wt1")
        wt2 = wp.tile([64, 576], bf16, tag="wt2")
        wdw = wp.tile([128, 45], f32, tag="wdw")
        pjt1 = wp.tile([128, 192], bf16, tag="pjt1")
        pjt2 = wp.tile([64, 192], bf16, tag="pjt2")
        ident = wp.tile([128, 128], bf16, tag="ident")
        ident64 = wp.tile([128, 64], bf16, tag="ident64")
        tempb = wp.tile([48, 4], f32, tag="tempb")
        for t, d in [(wt1, wt1_d), (wt2, wt2_d), (wdw, wdw_d), (pjt1, pjt1_d),
                     (pjt2, pjt2_d), (ident, id_d), (ident64, id64_d), (tempb, tmp_d)]:
            nc.sync.dma_start(t[:], d[:])

        # persistent accumulators
        ss_acc = stp.tile([128, 3 * NCH], f32, tag="ss")      # sumsq partials
        stats = stp.tile([128, 200], f32, tag="stats")        # pre-AR pack
        stats_rd = stp.tile([128, 200], f32, tag="stats_rd")  # post-AR
        g1 = ps_gram.tile([96, 192], f32, tag="g1")           # Gram c 0..95
        g2 = ps_gram.tile([96, 192], f32, tag="g2")           # Gram c 96..191
        vres = drp.tile([C, NPX], bf16, tag="vres")
        ar_in = drp.tile([128, 200], f32, tag="ar_in")
        ar_out = drp.tile([128, 200], f32, tag="ar_out")

        nc.vector.memset(stats[:], 0.0)

        # ================= main chunk loop =================
        for c in range(NCH):
            xoff = c * CR * WP
            xc1 = xp.tile([128, NF], bf16, tag="xc1")
            xc2 = xp.tile([64, NF], bf16, tag="xc2")
            nc.sync.dma_start(xc1[:], x_d[0:128, xoff:xoff + NF])
            nc.sync.dma_start(xc2[:], x_d[128:192, xoff:xoff + NF])

            cos = []                       # conv-out tiles for q/k o-tiles
            for j, (o0, orows) in enumerate(OT):
                # --- GEMM into psum pieces, evac to SBUF (ACT)
                qk = qkvp.tile([128, NF], bf16, tag="qk")
                p = 0
                while p < NF:
                    pw = min(512, NF - p)
                    ps = ps_g.tile([128, 512], f32, tag="gemm")
                    nc.tensor.matmul(ps[0:orows, 0:pw],
                                     wt1[:, o0:o0 + orows],
                                     xc1[:, p:p + pw], start=True, stop=False)
                    nc.tensor.matmul(ps[0:orows, 0:pw],
                                     wt2[:, o0:o0 + orows],
                                     xc2[:, p:p + pw], start=False, stop=True)
                    nc.scalar.copy(qk[0:orows, p:p + pw], ps[0:orows, 0:pw])
                    p += pw
                # --- odd-aligned sibling (shifted by one element) via DMA
                qs = qsp.tile([128, NF], bf16, tag="qs")
                nc.sync.dma_start(qs[0:orows, 0:NF - 2], qk[0:orows, 1:NF - 1])

                # --- depthwise 3x3: 1 tensor_scalar + 8 scalar_tensor_tensor
                co = cop.tile([128, NV], bf16, tag=f"co{j}")
                co3 = co[0:orows, :].rearrange("p (r c) -> p r c", c=W)
                qk3 = qk[0:orows, :].rearrange("p (r c) -> p r c", c=WP)
                qs3 = qs[0:orows, :].rearrange("p (r c) -> p r c", c=WP)
                for t, (dy, dx) in enumerate(TAPS):
                    r0 = 1 + dy
                    if dx == 0:
                        src = qk3[:, r0:r0 + CR, 2:2 + W]
                    else:
                        cs = 1 + dx  # shifted tile col: qk col (2+dx) - 1
                        src = qs3[:, r0:r0 + CR, cs:cs + W]
                    wsl = wdw[0:orows, 9 * j + t:9 * j + t + 1]
                    if t == 0:
                        nc.vector.tensor_scalar(co3[:], src, wsl, None, Alu.mult)
                    else:
                        nc.vector.scalar_tensor_tensor(
                            co3[:], src, wsl, co3[:], Alu.mult, Alu.add)

                if j >= 3:   # v channels -> DRAM scratch
                    nc.sync.dma_start(
                        vres[o0 - 384:o0 - 384 + orows, c * NV:(c + 1) * NV],
                        co[0:orows, :])
                else:        # q/k channels -> sumsq partial (ACT)
                    cos.append(co)
                    sq = sqp.tile([128, NV], bf16, tag="sq")
                    nc.scalar.activation(
                        sq[0:orows, :], co[0:orows, :], Act.Square,
                        accum_out=ss_acc[0:orows, j * NCH + c:j * NCH + c + 1])

            # --- transposes + Gram over this chunk's q/k
            co0, co1, co2 = cos
            for bb in range(0, PXB, 2):
                qt_ps = ps_t.tile([128, 384], bf16, tag="qt")
                kt_ps = ps_t.tile([128, 384], bf16, tag="kt")
                for s in range(2):
                    blk = (bb + s) * 128
                    ofs = 192 * s
                    nc.tensor.transpose(qt_ps[:, ofs:ofs + 128],
                                        co0[:, blk:blk + 128], ident[:])
                    nc.tensor.transpose(qt_ps[:, ofs + 128:ofs + 192],
                                        co1[0:64, blk:blk + 128],
                                        ident64[0:64, :])
                    nc.tensor.transpose(kt_ps[:, ofs:ofs + 64],
                                        co1[64:128, blk:blk + 128],
                                        ident64[64:128, :])
                    nc.tensor.transpose(kt_ps[:, ofs + 64:ofs + 192],
                                        co2[:, blk:blk + 128], ident[:])
                qt = qtp.tile([128, 384], bf16, tag="qts")
                kt = qtp.tile([128, 384], bf16, tag="kts")
                nc.scalar.copy(qt[:], qt_ps[:])
                nc.scalar.copy(kt[:], kt_ps[:])
                first = (c == 0 and bb == 0)
                last = (c == NCH - 1 and bb == PXB - 2)
                for s in range(2):
                    ofs = 192 * s
                    nc.tensor.matmul(g1[:], qt[:, ofs:ofs + 96],
                                     kt[:, ofs:ofs + 192],
                                     start=(first and s == 0),
                                     stop=(last and s == 1))
                    nc.tensor.matmul(g2[:], qt[:, ofs + 96:ofs + 192],
                                     kt[:, ofs:ofs + 192],
                                     start=(first and s == 0),
                                     stop=(last and s == 1))

        # ================= stats pack + AllReduce =================
        # Gram diagonal blocks -> stats[0:48, 48h:48h+48]
        gsb1 = stp.tile([96, 192], f32, tag="gsb1")
        gsb2 = stp.tile([96, 192], f32, tag="gsb2")
        nc.scalar.copy(gsb1[:], g1[:])
        nc.scalar.copy(gsb2[:], g2[:])
        nc.sync.dma_start(stats[0:48, 0:48], gsb1[0:48, 0:48])
        nc.sync.dma_start(stats[0:48, 48:96], gsb1[48:96, 48:96])
        nc.sync.dma_start(stats[0:48, 96:144], gsb2[0:48, 96:144])
        nc.sync.dma_start(stats[0:48, 144:192], gsb2[48:96, 144:192])
        for j in range(3):
            nc.vector.tensor_reduce(
                stats[0:OT[j][1], 192 + j:193 + j],
                ss_acc[0:OT[j][1], j * NCH:(j + 1) * NCH],
                mybir.AxisListType.X, Alu.add)
        nc.sync.dma_start(ar_in[:], stats[:])
        nc.gpsimd.collective_compute(
            "AllReduce", Alu.add,
            replica_groups=[[0, 1], [2, 3], [4, 5], [6, 7]],
            ins=[ar_in.opt()], outs=[ar_out.opt()])
        nc.sync.dma_start(stats_rd[:], ar_out[:])
        nc.sync.dma_start(dbg_d[:], stats_rd[:])

        # ================= softmax(attn) =================
        # reassemble qss/kss as [48 part, 4 head] via partition-moving DMAs
        qss = postp.tile([48, 4], f32, tag="qss")
        kss = postp.tile([48, 4], f32, tag="kss")
        mv = [
            (qss, 0, 0, 48, 192, 0), (qss, 1, 0, 48, 192, 48),
            (qss, 2, 0, 32, 192, 96), (qss, 2, 32, 16, 193, 0),
            (qss, 3, 0, 48, 193, 16),
            (kss, 0, 0, 48, 193, 64), (kss, 1, 0, 16, 193, 112),
            (kss, 1, 16, 32, 194, 0), (kss, 2, 0, 48, 194, 32),
            (kss, 3, 0, 48, 194, 80),
        ]
        for dst, h, dp, n, col, sp in mv:
            nc.sync.dma_start(dst[dp:dp + n, h:h + 1],
                              stats_rd[sp:sp + n, col:col + 1])
        rq = postp.tile([48, 4], f32, tag="rq")
        rk = postp.tile([48, 4], f32, tag="rk")
        nc.scalar.sqrt(rq[:], qss[:])
        nc.scalar.sqrt(rk[:], kss[:])
        nc.vector.reciprocal(rq[:], rq[:])
        nc.vector.reciprocal(rk[:], rk[:])
        nc.vector.tensor_tensor(rq[:], rq[:], tempb[:], Alu.mult)
        # rk as a [1,192] row (h-major): dst free idx 48h+d
        rk_row = postp.tile([1, 192], f32, tag="rkrow")
        for h in range(4):
            nc.sync.dma_start(rk_row[0:1, 48 * h:48 * h + 48], rk[:, h:h + 1])
        rk_row_b = postp.tile([1, 192], bf16, tag="rkrowb")
        nc.vector.tensor_copy(rk_row_b[:], rk_row[:])
        ones_f = postp.tile([1, 48], bf16, tag="onesf")
        nc.vector.memset(ones_f[:], 1.0)
        rk_bc = ps_post.tile([48, 192], f32, tag="post")
        nc.tensor.matmul(rk_bc[:], ones_f[:], rk_row_b[:],
                         start=True, stop=True)
        logits = postp.tile([48, 192], f32, tag="logits")
        for h in range(4):
            sl = slice(48 * h, 48 * h + 48)
            nc.vector.tensor_scalar(logits[:, sl], stats_rd[0:48, sl],
                                    rq[:, h:h + 1], None, Alu.mult)
        nc.vector.tensor_tensor(logits[:], logits[:], rk_bc[:], Alu.mult)
        l3 = logits[:].rearrange("p (h d) -> p h d", h=4)
        rmax = postp.tile([48, 4], f32, tag="rmax")
        nc.vector.tensor_reduce(rmax[:], l3, mybir.AxisListType.X, Alu.max)
        for h in range(4):
            sl = slice(48 * h, 48 * h + 48)
            nc.vector.tensor_scalar(logits[:, sl], logits[:, sl],
                                    rmax[:, h:h + 1], None, Alu.subtract)
        nc.scalar.activation(logits[:], logits[:], Act.Exp)
        rsum = postp.tile([48, 4], f32, tag="rsum")
        nc.vector.tensor_reduce(rsum[:], l3, mybir.AxisListType.X, Alu.add)
        nc.vector.reciprocal(rsum[:], rsum[:])
        attn = postp.tile([48, 192], bf16, tag="attn")
        for h in range(4):
            sl = slice(48 * h, 48 * h + 48)
            nc.vector.tensor_scalar(attn[:, sl], logits[:, sl],
                                    rsum[:, h:h + 1], None, Alu.mult)

        # ================= M_bT = BD^T @ projT =================
        # lhsT slot (48h+a, 48h+b) must hold attn_h[a, b]: copy attn directly
        bd1 = postp.tile([128, 192], bf16, tag="bd1")
        bd2 = postp.tile([64, 192], bf16, tag="bd2")
        nc.vector.memset(bd1[:], 0.0)
        nc.vector.memset(bd2[:], 0.0)
        nc.sync.dma_start(bd1[0:48, 0:48], attn[:, 0:48])
        nc.sync.dma_start(bd1[48:96, 48:96], attn[:, 48:96])
        nc.sync.dma_start(bd1[96:128, 96:144], attn[0:32, 96:144])
        nc.sync.dma_start(bd2[0:16, 96:144], attn[32:48, 96:144])
        nc.sync.dma_start(bd2[16:64, 144:192], attn[:, 144:192])
        mbt_ps1 = ps_post.tile([128, 192], f32, tag="post")
        nc.tensor.matmul(mbt_ps1[:], bd1[:, 0:128], pjt1[:], start=True, stop=False)
        nc.tensor.matmul(mbt_ps1[:], bd2[:, 0:128], pjt2[:], start=False, stop=True)
        mbt1 = postp.tile([128, 192], bf16, tag="mbt1")
        nc.scalar.copy(mbt1[:], mbt_ps1[:])
        mbt_ps2 = ps_post.tile([64, 192], f32, tag="post")
        nc.tensor.matmul(mbt_ps2[:], bd1[:, 128:192], pjt1[:], start=True, stop=False)
        nc.tensor.matmul(mbt_ps2[:], bd2[:, 128:192], pjt2[:], start=False, stop=True)
        mbt2 = postp.tile([64, 192], bf16, tag="mbt2")
        nc.scalar.copy(mbt2[:], mbt_ps2[:])

        # ================= y = M_b @ v =================
        for vc in range(NPX // VC):
            v1 = vcp.tile([128, VC], bf16, tag="v1")
            v2 = vcp.tile([64, VC], bf16, tag="v2")
            nc.sync.dma_start(v1[:], vres[0:128, vc * VC:(vc + 1) * VC])
            nc.sync.dma_start(v2[:], vres[128:192, vc * VC:(vc + 1) * VC])
            y1 = ysp.tile([128, VC], f32, tag="y1")
            y2 = ysp.tile([64, VC], f32, tag="y2")
            for p in range(0, VC, 512):
                ps = ps_g.tile([128, 512], f32, tag="gemm")
                nc.tensor.matmul(ps[:], mbt1[:, 0:128], v1[:, p:p + 512],
                                 start=True, stop=False)
                nc.tensor.matmul(ps[:], mbt2[:, 0:128], v2[:, p:p + 512],
                                 start=False, stop=True)
                nc.vector.tensor_copy(y1[:, p:p + 512], ps[:])
                ps2 = ps_g.tile([128, 512], f32, tag="gemm")
                nc.tensor.matmul(ps2[0:64, :], mbt1[:, 128:192], v1[:, p:p + 512],
                                 start=True, stop=False)
                nc.tensor.matmul(ps2[0:64, :], mbt2[:, 128:192], v2[:, p:p + 512],
                                 start=False, stop=True)
                nc.vector.tensor_copy(y2[:, p:p + 512], ps2[0:64, :])
            nc.sync.dma_start(y_d[0:128, vc * VC:(vc + 1) * VC], y1[:])
            nc.sync.dma_start(y_d[128:192, vc * VC:(vc + 1) * VC], y2[:])

    nc.compile()
    return nc


def _host_pack(x, qkv_w, dw_w, proj_w, temperature, H):
    HALF = H // 2
    SH_ROWS = HALF + 2
    bfa = lambda a: np.ascontiguousarray(a.astype(ml_dtypes.bfloat16))
    wT = qkv_w.T.astype(np.float32)                     # [192, 576]
    dw9 = dw_w.reshape(576, 9).astype(np.float32)
    wdw = np.zeros((128, 45), np.float32)
    for j, (o0, orows) in enumerate(OT):
        wdw[0:orows, 9 * j:9 * j + 9] = dw9[o0:o0 + orows]
    pjT = proj_w.T.astype(np.float32)
    shared = {
        "wt1": bfa(wT[0:128]), "wt2": bfa(wT[128:192]), "wdw": wdw,
        "pjt1": bfa(pjT[0:128]), "pjt2": bfa(pjT[128:192]),
        "ident": bfa(np.eye(128, dtype=np.float32)),
        "ident64": bfa(np.vstack([np.eye(64, dtype=np.float32)] * 2)),
        "tempb": np.ascontiguousarray(np.broadcast_to(
            np.asarray(temperature, np.float32).reshape(1, HEADS),
            (48, HEADS)).astype(np.float32)),
    }
    in_maps = []
    for core in range(N_CORES):
        b, h = core // 2, core % 2
        xs = np.zeros((C, SH_ROWS, WP), np.float32)
        r0 = h * HALF - 1
        lo, hi = max(r0, 0), min(r0 + SH_ROWS, H)
        xs[:, lo - r0:hi - r0, 2:2 + W] = x[b][:, lo:hi, :]
        in_maps.append({**shared, "x": bfa(xs.reshape(C, SH_ROWS * WP))})
    return in_maps


def kernel(x, qkv_w, dw_w, proj_w, temperature, num_heads):
    x = np.asarray(x, np.float32)
    H = x.shape[2]
    assert int(num_heads) == HEADS and x.shape == (B, C, H, W)
    key = (H,)
    if key not in _BUILT:
        _BUILT[key] = build(H=H, CR=16 if (H // 2) % 16 == 0 else H // 2)
    nc = _BUILT[key]
    in_maps = _host_pack(x, np.asarray(qkv_w, np.float32),
                         np.asarray(dw_w, np.float32),
                         np.asarray(proj_w, np.float32),
                         np.asarray(temperature, np.float32).reshape(-1), H)
    res = run_bass_kernel_spmd(nc, in_maps, list(range(N_CORES)))
    HALF = H // 2
    out = np.empty((B, C, H, W), np.float32)
    for core in range(N_CORES):
        b, h = core // 2, core % 2
        out[b, :, h * HALF:(h + 1) * HALF, :] = \
            res.results[core]["y"].reshape(C, HALF, W)
    return out


def build_empty(H=256):
    """Same external IO as build(), trivial body — for launch-overhead calibration."""
    HALF = H // 2
    SH_ROWS = HALF + 2
    NPX = HALF * W
    nc = bacc.Bacc("TRN2", target_bir_lowering=False, debug=False,
                   num_devices=N_CORES)
    dram = lambda n, s, d, kind: nc.dram_tensor(n, s, d, kind=kind).ap()
    x_d = dram("x", [C, SH_ROWS * WP], bf16, "ExternalInput")
    dram("wt1", [128, 576], bf16, "ExternalInput")
    dram("wt2", [64, 576], bf16, "ExternalInput")
    dram("wdw", [128, 45], f32, "ExternalInput")
    dram("pjt1", [128, 192], bf16, "ExternalInput")
    dram("pjt2", [64, 192], bf16, "ExternalInput")
    dram("ident", [128, 128], bf16, "ExternalInput")
    dram("ident64", [128, 64], bf16, "ExternalInput")
    dram("tempb", [48, 4], f32, "ExternalInput")
    dram("y", [C, NPX], f32, "ExternalOutput")
    dbg_d = dram("dbg", [128, 200], f32, "ExternalOutput")
    with tile.TileContext(nc) as tc, ExitStack() as ctx:
        sb = ctx.enter_context(tc.tile_pool(name="sb", bufs=1))
        t = sb.tile([128, 200], bf16)
        nc.sync.dma_start(t[:, 0:169], x_d[0:128, 0:169])
        t2 = sb.tile([128, 200], f32)
        nc.vector.tensor_copy(t2[:, 0:169], t[:, 0:169])
        nc.sync.dma_start(dbg_d[:, 0:169], t2[:, 0:169])
    nc.compile()
    return nc



# revision 23
# speedup vs baseline: 762.6475x; 762.6475x over previous
"""Trainium2 Bass kernel for Restormer-style transposed (channel) attention.

Math (per batch b):
  qkv = qkv_w @ x                    (1x1 conv, channel GEMM)
  qkv = DWConv3x3(qkv)               (per-channel 3x3, SAME zero pad)
  q,k,v = split(qkv); per head: q,k l2-normalized over spatial
  attn  = softmax((q_n @ k_n^T) * temperature)
  y     = proj_w @ (blockdiag(attn) @ v)

Sharding: 8 cores <-> (batch b = core//2, image-row half = core%2).
Cross-half reductions (channel sumsq + Gram) are tiny: one AllReduce of a
[128,200] f32 stat tile per pair. The final projection is folded with the
attention: y = (proj_w @ blockdiag(attn)) @ v, one 192x192 GEMM applied to v.

Engine layout per core (software-pipelined: chunk c's GEMM/conv overlaps
chunk c-1's transposes/Gram):
  PE : qkv GEMM, diag-matmul depthwise for PE_CONV o-tiles, q/k transposes,
       Gram, post matmuls, final GEMM
  DVE: depthwise conv tree (tensor_scalar 4x + tensor_tensor 2x) for DVE
       o-tiles, softmax small ops, half of final evac
  ACT: PSUM->SBUF evacuations, sumsq (Square+accum), exp
  DMA: x streaming (sync queue), shifted copy + v scratch (act queue), y store

Timing support: build(reps=N) wraps the whole body in a hardware loop so
test.py can measure (wall(N reps) - wall(1 rep)) / (N-1) = true HW ns/iter.
"""

import sys
sys.path.insert(0, '/opt/trn_rl_repo')

import numpy as np
import ml_dtypes
from contextlib import ExitStack

import concourse.bass as bass
import concourse.bacc as bacc
import concourse.tile as tile
import concourse.mybir as mybir
from concourse.bass_utils import run_bass_kernel_spmd

bf16 = mybir.dt.bfloat16
f32 = mybir.dt.float32
Alu = mybir.AluOpType
Act = mybir.ActivationFunctionType

B, C, HEADS, CPH = 4, 192, 4, 48
W = 256
WP = W + 4          # 2 zero cols left, 2 right (keeps 4B alignment)
N_CORES = 8

# o-tile partition map over the 576 qkv channels (q:0-191, k:192-383, v:384-575)
OT = [(0, 128), (128, 128), (256, 128), (384, 128), (512, 64)]
TAPS = [(dy, dx) for dy in (-1, 0, 1) for dx in (-1, 0, 1)]

# conv engine assignment: True -> depthwise for o-tile j runs on the tensor
# engine (9 accumulating diag matmuls per 512-px piece); False -> DVE tree.
PE_CONV = [False, True, False, True, True]
PE_SLOT = {}
for _j, _pe in enumerate(PE_CONV):
    if _pe:
        PE_SLOT[_j] = len(PE_SLOT)
N_PE_MATS = max(9 * len(PE_SLOT), 1)

USE_TREE = True
PIPELINED = True
QKT_SPLIT = True
GRAM_SPLIT = False

def set_pe_conv(lst):
    global PE_CONV, PE_SLOT, N_PE_MATS
    PE_CONV = list(lst)
    PE_SLOT = {}
    for j, pe in enumerate(PE_CONV):
        if pe:
            PE_SLOT[j] = len(PE_SLOT)
    N_PE_MATS = max(9 * len(PE_SLOT), 1)

_BUILT = {}


def build(H=256, CR=16, reps=1, trunc=0):
    """H: image height (256 real). CR: valid rows per chunk. reps: hw loop."""
    HALF = H // 2                      # image rows per core
    NCH = HALF // CR                   # chunks
    assert NCH * CR == HALF
    SH_ROWS = HALF + 2                 # shard rows incl. conv halo
    NF = (CR + 2) * WP                 # GEMM window elems per chunk
    NV = CR * W                        # valid conv-out elems per chunk
    NPX = HALF * W                     # valid pixels per core
    PXB = NV // 128                    # 128-px blocks per chunk
    assert PXB % 2 == 0
    assert CR % 2 == 0
    VC = 1024                          # final-GEMM v chunk cols
    assert NPX % VC == 0

    nc = bacc.Bacc("TRN2", target_bir_lowering=False, debug=False,
                   num_devices=N_CORES)
    dram = lambda n, s, d, kind: nc.dram_tensor(n, s, d, kind=kind).ap()
    x_d = dram("x", [C, SH_ROWS * WP], bf16, "ExternalInput")
    wt1_d = dram("wt1", [128, 576], bf16, "ExternalInput")
    wt2_d = dram("wt2", [64, 576], bf16, "ExternalInput")
    wdw_d = dram("wdw", [128, 45], f32, "ExternalInput")
    wdg_d = dram("wdg", [128, N_PE_MATS * 128], bf16, "ExternalInput")
    pjt1_d = dram("pjt1", [128, 192], bf16, "ExternalInput")
    pjt2_d = dram("pjt2", [64, 192], bf16, "ExternalInput")
    id_d = dram("ident", [128, 128], bf16, "ExternalInput")
    id64_d = dram("ident64", [128, 64], bf16, "ExternalInput")
    tmp_d = dram("tempb", [48, 4], f32, "ExternalInput")
    y_d = dram("y", [C, NPX], bf16, "ExternalOutput")
    dbg_d = dram("dbg", [128, 200], f32, "ExternalOutput")

    with tile.TileContext(nc) as tc, ExitStack() as ctx:
        P = lambda name, bufs, space="SBUF": ctx.enter_context(
            tc.tile_pool(name=name, bufs=bufs, space=space))
        wp = P("wp", 1)
        xp = P("xp", 2)
        qkvp = P("qkvp", 2)    # tags qkd (DVE units) / qkp (PE units)
        qsp = P("qsp", 2)
        cop = P("cop", 2)      # per-o-tile tags co0..co2 (q/k)
        vcop = P("vcop", 2)    # v conv-out pieces
        trp = P("trp", 1)      # DVE tree tmps
        qtp = P("qtp", 2)
        sqp = P("sqp", 1)
        stp = P("stp", 1)
        postp = P("postp", 1)
        vcp = P("vcp", 2)
        ysp = P("ysp", 2)
        ps_g = P("ps_g", 2, "PSUM")    # qkv GEMM pieces
        ps_c = P("ps_c", 2, "PSUM")    # PE-conv pieces
        ps_t = P("ps_t", 1 if QKT_SPLIT else 2, "PSUM")
        ps_gram = P("ps_gram", 1, "PSUM")
        drp = P("drp", 1, "DRAM")

        # ---- weights / constants in SBUF (outside the timing loop)
        wt1 = wp.tile([128, 576], bf16, tag="wt1")
        wt2 = wp.tile([64, 576], bf16, tag="wt2")
        wdw = wp.tile([128, 45], f32, tag="wdw")
        wdg = wp.tile([128, N_PE_MATS * 128], bf16, tag="wdg")
        pjt1 = wp.tile([128, 192], bf16, tag="pjt1")
        pjt2 = wp.tile([64, 192], bf16, tag="pjt2")
        ident = wp.tile([128, 128], bf16, tag="ident")
        ident64 = wp.tile([128, 64], bf16, tag="ident64")
        tempb = wp.tile([48, 4], f32, tag="tempb")
        for t, d in [(wt1, wt1_d), (wt2, wt2_d), (wdw, wdw_d), (wdg, wdg_d),
                     (pjt1, pjt1_d), (pjt2, pjt2_d), (ident, id_d),
                     (ident64, id64_d), (tempb, tmp_d)]:
            nc.sync.dma_start(t[:], d[:])

        def body():
            # persistent accumulators
            ss_acc = stp.tile([128, 3 * NCH], f32, tag="ss")      # sumsq partials
            stats = stp.tile([128, 200], f32, tag="stats")        # pre-AR pack
            stats_rd = stp.tile([128, 200], f32, tag="stats_rd")  # post-AR
            if GRAM_SPLIT:
                g1 = ps_gram.tile([96, 192], f32, tag="g1", name="g1")
                g2 = ps_gram.tile([96, 192], f32, tag="g2", name="g2")
            else:
                gram = ps_gram.tile([96, 1024], f32, tag="g")  # 2 banks: g1|g2
                g1 = gram[:, 0:192]
                g2 = gram[:, 512:704]
            vres = drp.tile([C, NPX], bf16, tag="vres")
            ar_in = drp.tile([128, 200], f32, tag="ar_in")
            ar_out = drp.tile([128, 200], f32, tag="ar_out")

            nc.vector.memset(stats[:], 0.0)

            def emit_gram(c, bb, qt, kt):
                first = (c == 0 and bb == 0)
                last = (c == NCH - 1 and bb == PXB - 2)
                for s in range(2):
                    ofs = 192 * s
                    nc.tensor.matmul(g1[:], qt[:, ofs:ofs + 96],
                                     kt[:, ofs:ofs + 192],
                                     start=(first and s == 0),
                                     stop=(last and s == 1))
                    nc.tensor.matmul(g2[:], qt[:, ofs + 96:ofs + 192],
                                     kt[:, ofs:ofs + 192],
                                     start=(first and s == 0),
                                     stop=(last and s == 1))

            def emit_chunk(c):
                """GEMM + depthwise for chunk c. Returns {j: co tile}."""
                xoff = c * CR * WP
                xc1 = xp.tile([128, NF], bf16, tag="xc1")
                xc2 = xp.tile([64, NF], bf16, tag="xc2")
                nc.sync.dma_start(xc1[:], x_d[0:128, xoff:xoff + NF])
                nc.sync.dma_start(xc2[:], x_d[128:192, xoff:xoff + NF])

                qks = {}
                # GEMM all o-tiles (PE dense), ACT evacs, stage qs copies
                for j, (o0, orows) in enumerate(OT):
                    qk = qkvp.tile([128, NF], bf16,
                                   tag="qkp" if PE_CONV[j] else "qkd")
                    qks[j] = qk
                    p = 0
                    while p < NF:
                        pw = min(512, NF - p)
                        ps = ps_g.tile([128, 512], f32, tag="gemm")
                        nc.tensor.matmul(ps[0:orows, 0:pw],
                                         wt1[:, o0:o0 + orows],
                                         xc1[:, p:p + pw], start=True, stop=False)
                        nc.tensor.matmul(ps[0:orows, 0:pw],
                                         wt2[:, o0:o0 + orows],
                                         xc2[:, p:p + pw], start=False, stop=True)
                        nc.scalar.copy(qk[0:orows, p:p + pw], ps[0:orows, 0:pw])
                        p += pw
                    if not PE_CONV[j]:
                        qs = qsp.tile([128, NF], bf16, tag="qs")
                        nc.sync.dma_start(qs[0:orows, 0:NF - 2],
                                            qk[0:orows, 1:NF - 1])
                        qks[j] = (qk, qs)

                cos = {}
                # PE diag-matmul depthwise
                for j, (o0, orows) in enumerate(OT):
                    if not PE_CONV[j]:
                        continue
                    qk = qks[j]
                    qk3 = qk[0:orows, :].rearrange("p (r c) -> p r c", c=WP)
                    slot = PE_SLOT[j]
                    co = None if j >= 3 else cop.tile([128, NV], bf16,
                                                      tag="co%d" % j)
                    for pp in range(CR // 2):
                        pc = ps_c.tile([128, 512], f32, tag="conv")
                        for t, (dy, dx) in enumerate(TAPS):
                            r0 = 2 * pp + 1 + dy
                            srcv = qk3[:, r0:r0 + 2, 2 + dx:2 + dx + W]
                            m0 = (9 * slot + t) * 128
                            dwm = wdg[0:orows, m0:m0 + orows]
                            nc.tensor.matmul(pc[0:orows, :], dwm, srcv,
                                             start=(t == 0), stop=(t == 8))
                        if j >= 3:
                            vco = vcop.tile([128, 512], bf16, tag="vco")
                            nc.scalar.copy(vco[0:orows, :], pc[0:orows, :])
                            nc.sync.dma_start(
                                vres[o0 - 384:o0 - 384 + orows,
                                     c * NV + pp * 512:c * NV + (pp + 1) * 512],
                                vco[0:orows, :])
                        else:
                            nc.scalar.copy(co[0:orows, pp * 512:(pp + 1) * 512],
                                           pc[0:orows, :])
                    if j < 3:
                        cos[j] = co

                # DVE tree depthwise
                for j, (o0, orows) in enumerate(OT):
                    if PE_CONV[j]:
                        continue
                    qk, qs = qks[j]
                    qk3 = qk[0:orows, :].rearrange("p (r c) -> p r c", c=WP)
                    qs3 = qs[0:orows, :].rearrange("p (r c) -> p r c", c=WP)
                    if j >= 3:
                        co = vcop.tile([128, NV], bf16, tag="vcof")
                    else:
                        co = cop.tile([128, NV], bf16, tag="co%d" % j)
                    co3 = co[0:orows, :].rearrange("p (r c) -> p r c", c=W)
                    ta = trp.tile([128, NV], bf16, tag="ta")
                    tb = trp.tile([128, NV], bf16, tag="tb")
                    ta3 = ta[0:orows, :].rearrange("p (r c) -> p r c", c=W)
                    tb3 = tb[0:orows, :].rearrange("p (r c) -> p r c", c=W)
                    for t, (dy, dx) in enumerate(TAPS):
                        r0 = 1 + dy
                        if dx == 0:
                            srcv = qk3[:, r0:r0 + CR, 2:2 + W]
                        else:
                            cs = 1 + dx  # shifted tile col: qk col (2+dx) - 1
                            srcv = qs3[:, r0:r0 + CR, cs:cs + W]
                        wsl = wdw[0:orows, 9 * j + t:9 * j + t + 1]
                        if not USE_TREE:
                            if t == 0:
                                nc.vector.tensor_scalar(co3[:], srcv, wsl, None, Alu.mult)
                            else:
                                nc.vector.scalar_tensor_tensor(
                                    co3[:], srcv, wsl, co3[:], Alu.mult, Alu.add)
                            continue
                        dst = ta3 if t == 0 else tb3
                        nc.vector.tensor_scalar(dst[:], srcv, wsl, None, Alu.mult)
                        if t > 0:
                            out = co[0:orows, :] if t == 8 else ta[0:orows, :]
                            nc.vector.tensor_tensor(
                                out, ta[0:orows, :], tb[0:orows, :], Alu.add)
                    if j >= 3:
                        nc.sync.dma_start(
                            vres[o0 - 384:o0 - 384 + orows, c * NV:(c + 1) * NV],
                            co[0:orows, :])
                    else:
                        cos[j] = co
                return cos

            def emit_tail(c, cos):
                """Transposes + Gram + sumsq for chunk c (runs during c+1)."""
                co0, co1, co2 = cos[0], cos[1], cos[2]
                prev = None
                for bb in range(0, PXB, 2):
                    if QKT_SPLIT:
                        qt_ps = ps_t.tile([128, 384], bf16, tag="qtp2", name="qtps")
                        kt_ps = ps_t.tile([128, 384], bf16, tag="ktp2", name="ktps")
                    else:
                        qkt_ps = ps_t.tile([128, 768], bf16, tag="qkt")
                        qt_ps = qkt_ps[:, 0:384]
                        kt_ps = qkt_ps[:, 384:768]
                    for s in range(2):
                        blk = (bb + s) * 128
                        ofs = 192 * s
                        nc.tensor.transpose(qt_ps[:, ofs:ofs + 128],
                                            co0[:, blk:blk + 128], ident[:])
                        nc.tensor.transpose(qt_ps[:, ofs + 128:ofs + 192],
                                            co1[0:64, blk:blk + 128],
                                            ident64[0:64, :])
                        nc.tensor.transpose(kt_ps[:, ofs:ofs + 64],
                                            co1[64:128, blk:blk + 128],
                                            ident64[64:128, :])
                        nc.tensor.transpose(kt_ps[:, ofs + 64:ofs + 192],
                                            co2[:, blk:blk + 128], ident[:])
                    qkt = qtp.tile([128, 768], bf16, tag="qkts")
                    if QKT_SPLIT:
                        nc.scalar.copy(qkt[:, 0:384], qt_ps[:])
                        nc.scalar.copy(qkt[:, 384:768], kt_ps[:])
                    else:
                        nc.scalar.copy(qkt[:], qkt_ps[:])
                    qt = qkt[:, 0:384]
                    kt = qkt[:, 384:768]
                    if prev is not None:
                        emit_gram(*prev)
                    prev = (c, bb, qt, kt)
                emit_gram(*prev)
                for j, (o0, orows) in enumerate(OT[:3]):
                    sq = sqp.tile([128, NV], bf16, tag="sq")
                    nc.scalar.activation(
                        sq[0:orows, :], cos[j][0:orows, :], Act.Square,
                        accum_out=ss_acc[0:orows, j * NCH + c:j * NCH + c + 1])

            # ============ pipelined main loop ============
            if PIPELINED:
                pend = None
                for c in range(NCH):
                    cos = emit_chunk(c)
                    if pend is not None:
                        emit_tail(*pend)
                    pend = (c, cos)
                emit_tail(*pend)
            else:
                for c in range(NCH):
                    cos = emit_chunk(c)
                    emit_tail(c, cos)

            if trunc == 1:
                return
            # ================= stats pack + AllReduce =================
            gsb1 = stp.tile([96, 192], f32, tag="gsb1")
            gsb2 = stp.tile([96, 192], f32, tag="gsb2")
            nc.scalar.copy(gsb1[:], g1[:])
            nc.scalar.copy(gsb2[:], g2[:])
            nc.sync.dma_start(stats[0:48, 0:48], gsb1[0:48, 0:48])
            nc.sync.dma_start(stats[0:48, 48:96], gsb1[48:96, 48:96])
            nc.sync.dma_start(stats[0:48, 96:144], gsb2[0:48, 96:144])
            nc.sync.dma_start(stats[0:48, 144:192], gsb2[48:96, 144:192])
            for j in range(3):
                nc.vector.tensor_reduce(
                    stats[0:OT[j][1], 192 + j:193 + j],
                    ss_acc[0:OT[j][1], j * NCH:(j + 1) * NCH],
                    mybir.AxisListType.X, Alu.add)
            nc.sync.dma_start(ar_in[:], stats[:])
            nc.gpsimd.collective_compute(
                "AllReduce", Alu.add,
                replica_groups=[[0, 1], [2, 3], [4, 5], [6, 7]],
                ins=[ar_in.opt()], outs=[ar_out.opt()])
            nc.sync.dma_start(stats_rd[:], ar_out[:])
            nc.sync.dma_start(dbg_d[:], stats_rd[:])

            if trunc == 2:
                return
            # ================= softmax(attn) =================
            qss = postp.tile([48, 4], f32, tag="qss")
            kss = postp.tile([48, 4], f32, tag="kss")
            mv = [
                (qss, 0, 0, 48, 192, 0), (qss, 1, 0, 48, 192, 48),
                (qss, 2, 0, 32, 192, 96), (qss, 2, 32, 16, 193, 0),
                (qss, 3, 0, 48, 193, 16),
                (kss, 0, 0, 48, 193, 64), (kss, 1, 0, 16, 193, 112),
                (kss, 1, 16, 32, 194, 0), (kss, 2, 0, 48, 194, 32),
                (kss, 3, 0, 48, 194, 80),
            ]
            for dst, h, dp, n, col, sp in mv:
                nc.sync.dma_start(dst[dp:dp + n, h:h + 1],
                                  stats_rd[sp:sp + n, col:col + 1])
            rq = postp.tile([48, 4], f32, tag="rq")
            rk = postp.tile([48, 4], f32, tag="rk")
            nc.scalar.sqrt(rq[:], qss[:])
            nc.scalar.sqrt(rk[:], kss[:])
            nc.vector.reciprocal(rq[:], rq[:])
            nc.vector.reciprocal(rk[:], rk[:])
            nc.vector.tensor_tensor(rq[:], rq[:], tempb[:], Alu.mult)
            rk_row = postp.tile([1, 192], f32, tag="rkrow")
            for h in range(4):
                nc.sync.dma_start(rk_row[0:1, 48 * h:48 * h + 48], rk[:, h:h + 1])
            rk_row_b = postp.tile([1, 192], bf16, tag="rkrowb")
            nc.vector.tensor_copy(rk_row_b[:], rk_row[:])
            ones_f = postp.tile([1, 48], bf16, tag="onesf")
            nc.vector.memset(ones_f[:], 1.0)
            rk_bc_t = ps_c.tile([48, 512], f32, tag="conv", name="rkbc")
            rk_bc = rk_bc_t[:, 0:192]
            nc.tensor.matmul(rk_bc[:], ones_f[:], rk_row_b[:],
                             start=True, stop=True)
            logits = postp.tile([48, 192], f32, tag="logits")
            for h in range(4):
                sl = slice(48 * h, 48 * h + 48)
                nc.vector.tensor_scalar(logits[:, sl], stats_rd[0:48, sl],
                                        rq[:, h:h + 1], None, Alu.mult)
            nc.vector.tensor_tensor(logits[:], logits[:], rk_bc[:], Alu.mult)
            l3 = logits[:].rearrange("p (h d) -> p h d", h=4)
            rmax = postp.tile([48, 4], f32, tag="rmax")
            nc.vector.tensor_reduce(rmax[:], l3, mybir.AxisListType.X, Alu.max)
            for h in range(4):
                sl = slice(48 * h, 48 * h + 48)
                nc.vector.tensor_scalar(logits[:, sl], logits[:, sl],
                                        rmax[:, h:h + 1], None, Alu.subtract)
            nc.scalar.activation(logits[:], logits[:], Act.Exp)
            rsum = postp.tile([48, 4], f32, tag="rsum")
            nc.vector.tensor_reduce(rsum[:], l3, mybir.AxisListType.X, Alu.add)
            nc.vector.reciprocal(rsum[:], rsum[:])
            attn = postp.tile([48, 192], bf16, tag="attn")
            for h in range(4):
                sl = slice(48 * h, 48 * h + 48)
                nc.vector.tensor_scalar(attn[:, sl], logits[:, sl],
                                        rsum[:, h:h + 1], None, Alu.mult)

            # ================= M_bT = BD^T @ projT =================
            bd1 = postp.tile([128, 192], bf16, tag="bd1")
            bd2 = postp.tile([64, 192], bf16, tag="bd2")
            nc.vector.memset(bd1[:], 0.0)
            nc.vector.memset(bd2[:], 0.0)
            nc.sync.dma_start(bd1[0:48, 0:48], attn[:, 0:48])
            nc.sync.dma_start(bd1[48:96, 48:96], attn[:, 48:96])
            nc.sync.dma_start(bd1[96:128, 96:144], attn[0:32, 96:144])
            nc.sync.dma_start(bd2[0:16, 96:144], attn[32:48, 96:144])
            nc.sync.dma_start(bd2[16:64, 144:192], attn[:, 144:192])
            mbt_ps1_t = ps_g.tile([128, 512], f32, tag="gemm", name="mbtp1")
            mbt_ps1 = mbt_ps1_t[:, 0:192]
            nc.tensor.matmul(mbt_ps1[:], bd1[:, 0:128], pjt1[:], start=True, stop=False)
            nc.tensor.matmul(mbt_ps1[:], bd2[:, 0:128], pjt2[:], start=False, stop=True)
            mbt1 = postp.tile([128, 192], bf16, tag="mbt1")
            nc.scalar.copy(mbt1[:], mbt_ps1[:])
            mbt_ps2_t = ps_c.tile([64, 512], f32, tag="conv", name="mbtp2")
            mbt_ps2 = mbt_ps2_t[:, 0:192]
            nc.tensor.matmul(mbt_ps2[:], bd1[:, 128:192], pjt1[:], start=True, stop=False)
            nc.tensor.matmul(mbt_ps2[:], bd2[:, 128:192], pjt2[:], start=False, stop=True)
            mbt2 = postp.tile([64, 192], bf16, tag="mbt2")
            nc.scalar.copy(mbt2[:], mbt_ps2[:])

            if trunc == 3:
                return
            # ================= y = M_b @ v =================
            for vc in range(NPX // VC):
                v1 = vcp.tile([128, VC], bf16, tag="v1")
                v2 = vcp.tile([64, VC], bf16, tag="v2")
                nc.sync.dma_start(v1[:], vres[0:128, vc * VC:(vc + 1) * VC])
                nc.sync.dma_start(v2[:], vres[128:192, vc * VC:(vc + 1) * VC])
                y1 = ysp.tile([128, VC], bf16, tag="y1")
                y2 = ysp.tile([64, VC], bf16, tag="y2")
                for p in range(0, VC, 512):
                    ps = ps_g.tile([128, 512], f32, tag="gemm")
                    nc.tensor.matmul(ps[:], mbt1[:, 0:128], v1[:, p:p + 512],
                                     start=True, stop=False)
                    nc.tensor.matmul(ps[:], mbt2[:, 0:128], v2[:, p:p + 512],
                                     start=False, stop=True)
                    nc.scalar.copy(y1[:, p:p + 512], ps[:])
                    ps2 = ps_c.tile([128, 512], f32, tag="conv")
                    nc.tensor.matmul(ps2[0:64, :], mbt1[:, 128:192],
                                     v1[:, p:p + 512], start=True, stop=False)
                    nc.tensor.matmul(ps2[0:64, :], mbt2[:, 128:192],
                                     v2[:, p:p + 512], start=False, stop=True)
                    nc.vector.tensor_copy(y2[:, p:p + 512], ps2[0:64, :])
                nc.sync.dma_start(y_d[0:128, vc * VC:(vc + 1) * VC], y1[:])
                nc.sync.dma_start(y_d[128:192, vc * VC:(vc + 1) * VC], y2[:])

        if reps == 1:
            body()
        else:
            with tc.For_i(0, reps):
                body()

    nc.compile()
    return nc


def _host_pack(x, qkv_w, dw_w, proj_w, temperature, H):
    HALF = H // 2
    SH_ROWS = HALF + 2
    bfa = lambda a: np.ascontiguousarray(a.astype(ml_dtypes.bfloat16))
    wT = qkv_w.T.astype(np.float32)                     # [192, 576]
    dw9 = dw_w.reshape(576, 9).astype(np.float32)
    wdw = np.zeros((128, 45), np.float32)
    wdg = np.zeros((128, N_PE_MATS * 128), np.float32)
    for j, (o0, orows) in enumerate(OT):
        wdw[0:orows, 9 * j:9 * j + 9] = dw9[o0:o0 + orows]
        if j in PE_SLOT:
            for t in range(9):
                col = (9 * PE_SLOT[j] + t) * 128
                wdg[0:orows, col:col + orows] = np.diag(dw9[o0:o0 + orows, t])
    pjT = proj_w.T.astype(np.float32)
    shared = {
        "wt1": bfa(wT[0:128]), "wt2": bfa(wT[128:192]), "wdw": wdw,
        "wdg": bfa(wdg),
        "pjt1": bfa(pjT[0:128]), "pjt2": bfa(pjT[128:192]),
        "ident": bfa(np.eye(128, dtype=np.float32)),
        "ident64": bfa(np.vstack([np.eye(64, dtype=np.float32)] * 2)),
        "tempb": np.ascontiguousarray(np.broadcast_to(
            np.asarray(temperature, np.float32).reshape(1, HEADS),
            (48, HEADS)).astype(np.float32)),
    }
    in_maps = []
    for core in range(N_CORES):
        b, h = core // 2, core % 2
        xs = np.zeros((C, SH_ROWS, WP), np.float32)
        r0 = h * HALF - 1
        lo, hi = max(r0, 0), min(r0 + SH_ROWS, H)
        xs[:, lo - r0:hi - r0, 2:2 + W] = x[b][:, lo:hi, :]
        in_maps.append({**shared, "x": bfa(xs.reshape(C, SH_ROWS * WP))})
    return in_maps


def kernel(x, qkv_w, dw_w, proj_w, temperature, num_heads):
    x = np.asarray(x, np.float32)
    H = x.shape[2]
    assert int(num_heads) == HEADS and x.shape == (B, C, H, W)
    key = (H, 1)
    if key not in _BUILT:
        _BUILT[key] = build(H=H, CR=16 if (H // 2) % 16 == 0 else H // 2)
    nc = _BUILT[key]
    in_maps = _host_pack(x, np.asarray(qkv_w, np.float32),
                         np.asarray(dw_w, np.float32),
                         np.asarray(proj_w, np.float32),
                         np.asarray(temperature, np.float32).reshape(-1), H)
    res = run_bass_kernel_spmd(nc, in_maps, list(range(N_CORES)))
    HALF = H // 2
    out = np.empty((B, C, H, W), np.float32)
    for core in range(N_CORES):
        b, h = core // 2, core % 2
        out[b, :, h * HALF:(h + 1) * HALF, :] = \
            res.results[core]["y"].astype(np.float32).reshape(C, HALF, W)
    return out
